# revision 1
# baseline (speedup 1.0000x reference)
"""Trainium2 Bass kernel for nn_AttentionV1 (spatial attention block).

Reference computation (per batch b):
    q = wq @ x + bq            [128, 4096]
    k = wk @ aux + bk          [128, 4096]
    v = wv @ x + bv            [128, 4096]
    s = k^T q                  [4096 k, 4096 q]
    a = softmax(s, axis=q)     (normalize across QUERIES for each key row)
    y = v @ a                  [128, 4096]
    z = wz @ y + bz + x        [256, 4096]

Sharding: 8 cores = 4 batches x 2 query-halves.  Each core owns 2048 query
columns of one batch and computes K / V^T for all 4096 keys.  The softmax
normalization axis (q) is sharded, so per 512-key chunk the two cores of a
pair AllReduce their partial exp-rowsums (a 4 KB message) and fold 1/rowsum
into the V^T rows before the y matmuls.  y accumulates in PSUM across all
8 key chunks; the output projection + residual is computed per query half,
so no large tensors ever cross cores.
"""

import sys

if "/opt/trn_rl_repo" not in sys.path:
    sys.path.insert(0, "/opt/trn_rl_repo")

import numpy as np

import concourse.bass as bass  # noqa: F401  (import keeps bass registered)
import concourse.mybir as mybir
import concourse.tile as tile
from concourse import bacc
from concourse import bass2jax
from concourse.masks import make_identity

F32 = mybir.dt.float32
F32R = mybir.dt.float32r
AF = mybir.ActivationFunctionType
ALU = mybir.AluOpType

# Problem constants (hardcoded per harness contract).
B, C = 4, 256
CH = 128          # C // 2, the qkv channel count == SBUF partition count
N = 4096          # H * W
NQ = 2048         # query columns per core (N / 2)
NCORES = 8
NCHUNK = 8        # key chunks
NSUB = 4          # 128-row subchunks per key chunk
QT = 512          # matmul moving-dim tile
EXP_BIAS = -40.0  # constant shift inside exp() to avoid fp32 overflow

# Matmul precision knobs: float32r runs the PE at 4x the fp32 rate for
# 512-wide moving operands.  Attention = the S and y matmuls; proj = Q/K/z.
F32R_ATTN = True
F32R_PROJ = True

GROUPS = [[0, 1], [2, 3], [4, 5], [6, 7]]


def build_program(f32r_attn: bool = F32R_ATTN, f32r_proj: bool = F32R_PROJ,
                  reps: int = 1, skip_ar: bool = False, stage: str = "full"):
    assert stage == "full"
    nc = bacc.Bacc("TRN2", target_bir_lowering=False, debug=False,
                   num_devices=NCORES)

    DTA = F32R if f32r_attn else F32   # attention-path matmul dtype
    DTP = F32R if f32r_proj else F32   # projection-path matmul dtype

    x_d = nc.dram_tensor("x", [C, N], DTP, kind="ExternalInput")
    xq_d = nc.dram_tensor("xq", [C, NQ], DTP, kind="ExternalInput")
    aux_d = nc.dram_tensor("aux", [C, N], DTP, kind="ExternalInput")
    wqT_d = nc.dram_tensor("wqT", [C, CH], DTP, kind="ExternalInput")
    wkT_d = nc.dram_tensor("wkT", [C, CH], DTP, kind="ExternalInput")
    wvT_d = nc.dram_tensor("wvT", [C, CH], DTP, kind="ExternalInput")
    wzT_d = nc.dram_tensor("wzT", [CH, C], DTA, kind="ExternalInput")
    bq_d = nc.dram_tensor("bq", [CH, 1], F32, kind="ExternalInput")
    bk_d = nc.dram_tensor("bk", [CH, 1], F32, kind="ExternalInput")
    bv_d = nc.dram_tensor("bv", [CH, 1], F32, kind="ExternalInput")
    bz_d = nc.dram_tensor("bz", [C, 1], F32, kind="ExternalInput")
    z_d = nc.dram_tensor("z", [C, NQ], F32, kind="ExternalOutput")

    with tile.TileContext(nc) as tc:
        for rep in range(reps):
            with (
                tc.tile_pool(name="const", bufs=1) as constp,
                tc.tile_pool(name="persist", bufs=1) as persist,
                tc.tile_pool(name="dram", bufs=NCHUNK, space="DRAM") as dramp,
            ):
                # Collective warm-up: a tiny AllReduce issued first absorbs
                # the first-collective setup + core-start skew while the
                # input DMAs and projections proceed.  gpsimd carries only
                # collectives (its queue blocks on their completion).
                warm_sb = constp.tile([1, 4], F32, tag="warm", name="warm_sb")
                nc.vector.memset(warm_sb[:], 1.0)
                ident_early = True  # identity built below, before the AR
                warm_in = dramp.tile([1, 4], F32, tag="warmin", name="warmin")
                warm_out = dramp.tile([1, 4], F32, tag="warmout",
                                      name="warmout")
                nc.sync.dma_start(warm_in[:], warm_sb[:])
                nc.gpsimd.collective_compute(
                    "AllReduce", ALU.add, replica_groups=GROUPS,
                    ins=[warm_in.opt()], outs=[warm_out.opt()])
                warm_back = constp.tile([1, 4], F32, tag="warmb",
                                        name="warm_back")
                nc.sync.dma_start(warm_back[:], warm_out[:])

                # ---- constant tiles ----
                wqT = [constp.tile([128, CH], DTP, tag=f"wq{i}", name=f"wq{i}")
                       for i in range(2)]
                wkT = [constp.tile([128, CH], DTP, tag=f"wk{i}", name=f"wk{i}")
                       for i in range(2)]
                wvT = [constp.tile([128, CH], DTP, tag=f"wv{i}", name=f"wv{i}")
                       for i in range(2)]
                wzT_sb = constp.tile([128, C], DTA, tag="wz", name="wzT_sb")
                bq_sb = constp.tile([CH, 1], F32, tag="bq", name="bq_sb")
                bk_sb = constp.tile([CH, 1], F32, tag="bk", name="bk_sb")
                bz_sb = [constp.tile([128, 1], F32, tag=f"bz{i}",
                                     name=f"bz{i}") for i in range(2)]
                bv_sb = constp.tile([CH, 1], F32, tag="bv", name="bv_sb")
                ebias = constp.tile([128, 1], F32, tag="ebias", name="ebias")
                nc.vector.memset(ebias[:], EXP_BIAS)
                ident0 = constp.tile([128, 128], F32, tag="ident0",
                                     name="ident0")
                make_identity(nc, ident0[:])
                ident = constp.tile([128, 128], F32R, tag="ident",
                                    name="ident")
                nc.vector.tensor_copy(ident[:], ident0[:])

                # ---- persistent activations ----
                xq_sb = [persist.tile([128, NQ], DTP, tag=f"xq{i}",
                                      name=f"xq{i}") for i in range(2)]
                K_sb = persist.tile([128, N], DTA, tag="K", name="K_sb")
                Q_sb = persist.tile([128, NQ], DTA, tag="Q", name="Q_sb")
                Vt = [persist.tile([128, CH], DTA, tag=f"vt{g}",
                                   name=f"vt{g}") for g in range(32)]
                y_sb = [persist.tile([128, QT], DTA, tag=f"y{qt}",
                                     name=f"ysb{qt}") for qt in range(4)]

                # PSUM pools alive for the whole compute region:
                #   sps  4 banks (score strips; also z matmuls at the end)
                #   vtp  2 banks (V^T projection tiles)
                # plus 2 banks for the projection pool early on, then the
                # rotating y accumulator once projections are done.
                sps_cm = tc.tile_pool(name="s_ps", bufs=2, space="PSUM")
                sps = sps_cm.__enter__()
                vtp_cm = tc.tile_pool(name="vt_ps", bufs=2, space="PSUM")
                vtp = vtp_cm.__enter__()

                # ---- projections: K cols 0:2048 and all of Q ----
                with (
                    tc.tile_pool(name="xaux", bufs=1) as xauxp,
                    tc.tile_pool(name="pj_ps", bufs=2, space="PSUM") as pjps,
                ):
                    aux_sb = [xauxp.tile([128, N // 2], DTP, tag=f"a{i}",
                                         name=f"a{i}") for i in range(2)]
                    # sync-queue load order == arrival priority
                    for i in range(2):
                        nc.sync.dma_start(wkT[i][:],
                                          wkT_d[i * 128:(i + 1) * 128, :])
                    for i in range(2):
                        nc.sync.dma_start(wqT[i][:],
                                          wqT_d[i * 128:(i + 1) * 128, :])
                    for i in range(2):
                        nc.sync.dma_start(xq_sb[i][:],
                                          xq_d[i * 128:(i + 1) * 128, :])
                    for i in range(2):
                        nc.sync.dma_start(aux_sb[i][:, 0:512],
                                          aux_d[i * 128:(i + 1) * 128,
                                                0:512])
                    for i in range(2):
                        nc.sync.dma_start(aux_sb[i][:, 512:2048],
                                          aux_d[i * 128:(i + 1) * 128,
                                                512:2048])
                    nc.sync.dma_start(bk_sb[:], bk_d[:, :])
                    nc.sync.dma_start(bq_sb[:], bq_d[:, :])
                    nc.sync.dma_start(bv_sb[:], bv_d[:, :])
                    for i in range(2):
                        nc.sync.dma_start(wvT[i][:],
                                          wvT_d[i * 128:(i + 1) * 128, :])
                    nc.sync.dma_start(wzT_sb[:], wzT_d[:, :])
                    for i in range(2):
                        nc.sync.dma_start(bz_sb[i][:],
                                          bz_d[i * 128:(i + 1) * 128, :])

                    def proj_group(dst, w01, src01, bias, tiles, src_base):
                        # each ci half-pass keeps the stationary operand
                        # constant so LDWEIGHTS stays hidden
                        pss = []
                        for t in tiles:
                            ps = pjps.tile([128, QT], F32, tag="pj",
                                           name="pjps")
                            dsl = slice(t * QT, (t + 1) * QT)
                            ssl = slice(t * QT - src_base,
                                        (t + 1) * QT - src_base)
                            nc.tensor.matmul(ps[:], w01[0][:],
                                             src01[0][:, ssl],
                                             start=True, stop=False)
                            pss.append((ps, dsl, ssl))
                        for ps, dsl, ssl in pss:
                            nc.tensor.matmul(ps[:], w01[1][:],
                                             src01[1][:, ssl],
                                             start=False, stop=True)
                        for ps, dsl, ssl in pss:
                            nc.vector.tensor_scalar_add(dst[:, dsl], ps[:],
                                                        bias[:])

                    proj_group(K_sb, wkT, aux_sb, bk_sb, [0], 0)
                    for grp in range(2):
                        proj_group(Q_sb, wqT, xq_sb, bq_sb,
                                   [grp * 2, grp * 2 + 1], 0)
                    proj_group(K_sb, wkT, aux_sb, bk_sb, [1], 0)
                    proj_group(K_sb, wkT, aux_sb, bk_sb, [2, 3], 0)

                # ---- stage 1: attention ----
                # Rowsum AllReduce groups: chunks (0,1),(2,3),(4,5),(6),(7).
                with (
                    tc.tile_pool(name="E", bufs=12) as Ep,
                    tc.tile_pool(name="rp", bufs=4) as rp,
                    tc.tile_pool(name="xc", bufs=2) as xcp,
                    tc.tile_pool(name="y_ps", bufs=2, space="PSUM") as yps,
                ):
                    pend = []
                    pending_y = []
                    late_k = []
                    group_idx = 0
                    for kc in range(NCHUNK):
                        E = [Ep.tile([128, NQ], DTA, tag="E", name="Etile")
                             for _ in range(NSUB)]
                        r_part = rp.tile([128, 8], F32, tag="rpart",
                                         name="rpart")
                        for s in range(NSUB):
                            ksl = slice((kc * NSUB + s) * 128,
                                        (kc * NSUB + s + 1) * 128)
                            for st in range(2):
                                ps = sps.tile([128, 1024], F32, tag="s",
                                              name="sps")
                                for hh in range(2):
                                    qsl = slice(st * 1024 + hh * QT,
                                                st * 1024 + (hh + 1) * QT)
                                    nc.tensor.matmul(
                                        ps[:, hh * QT:(hh + 1) * QT],
                                        K_sb[:, ksl], Q_sb[:, qsl],
                                        start=True, stop=True)
                                # e = exp(s+EXP_BIAS); accum_out = rowsum(e)
                                col = st * NSUB + s
                                nc.scalar.activation(
                                    E[s][:, st * 1024:(st + 1) * 1024],
                                    ps[:], AF.Exp, bias=ebias[:], scale=1.0,
                                    accum_out=r_part[:, col:col + 1])
                                # drain deferred y matmuls between strips so
                                # the PE never runs a long y-only phase that
                                # starves the exp pipeline
                                if s * 2 + st >= 3:
                                    for _ in range(5):
                                        if pending_y:
                                            pending_y.pop(0)()
                                if not pending_y:
                                    # keep the PE activity monitor warm so
                                    # matmuls run at 2.4 GHz: one throwaway
                                    # matmul per strip during ACT-paced
                                    # phases
                                    wps = vtp.tile([128, 512], F32, tag="v",
                                                   name="warmmm")
                                    nc.tensor.matmul(wps[:], K_sb[:, 0:128],
                                                     Q_sb[:, 0:QT],
                                                     start=True, stop=True)
                        # V^T tiles for this chunk (x re-streamed per chunk)
                        xcs = [xcp.tile([128, 512], DTP, tag=f"xc{i}",
                                        name=f"xc{i}") for i in range(2)]
                        for i in range(2):
                            nc.sync.dma_start(
                                xcs[i][:],
                                x_d[i * 128:(i + 1) * 128,
                                    kc * 512:(kc + 1) * 512])
                        # V[c, k] for this chunk in one wide matmul
                        # pair, then per-sub PE transposes to V^T[k, c]
                        vpsV = vtp.tile([128, 512], F32, tag="v", name="vpsV")
                        nc.tensor.matmul(vpsV[:], wvT[0][:], xcs[0][:],
                                         start=True, stop=False)
                        nc.tensor.matmul(vpsV[:], wvT[1][:], xcs[1][:],
                                         start=False, stop=True)
                        vchunk = xcp.tile([128, 512], DTA, tag="vsb",
                                          name="vchunk")
                        nc.vector.tensor_scalar_add(vchunk[:], vpsV[:],
                                                    bv_sb[:])
                        for s in range(NSUB):
                            g = kc * NSUB + s
                            tps = vtp.tile([128, 512], F32, tag="v",
                                           name="tps")
                            nc.tensor.transpose(
                                tps[:, 0:CH].bitcast(F32R),
                                vchunk[:, s * 128:(s + 1) * 128],
                                ident[:])
                            nc.vector.tensor_copy(Vt[g][:], tps[:, 0:CH])
                        if kc < 4:
                            # K column tile kc+4 (needed at chunk kc+4):
                            # stage the aux piece now, matmul next chunk
                            t = kc + 4
                            sl = slice(t * QT, (t + 1) * QT)
                            axs = [xcp.tile([128, 512], DTP, tag=f"ax{i}",
                                            name=f"ax{i}") for i in range(2)]
                            for i in range(2):
                                nc.sync.dma_start(
                                    axs[i][:],
                                    aux_d[i * 128:(i + 1) * 128, sl])
                            late_k.append((t, sl, axs))
                        if late_k and late_k[0][0] <= kc + 3:
                            t, sl, axs = late_k.pop(0)
                            kps = vtp.tile([128, 512], F32, tag="v",
                                           name="kps")
                            nc.tensor.matmul(kps[:], wkT[0][:], axs[0][:],
                                             start=True, stop=False)
                            nc.tensor.matmul(kps[:], wkT[1][:], axs[1][:],
                                             start=False, stop=True)
                            nc.vector.tensor_scalar_add(
                                K_sb[:, sl], kps[:], bk_sb[:])
                        pend.append((kc, E, r_part))
                        if not ((kc % 2 == 1 and kc < 6) or kc >= 6):
                            continue
                        n = len(pend)
                        # complete the rowsums across the query-half pair
                        if skip_ar:
                            rsrcs = [p[2] for p in pend]
                        else:
                            rin = dramp.tile([128, 8 * n], F32, tag="rin",
                                             name="rin")
                            rout = dramp.tile([128, 8 * n], F32, tag="rout",
                                              name="rout")
                            for j, (_, _, rp_j) in enumerate(pend):
                                nc.gpsimd.dma_start(rin[:, 8 * j:8 * j + 8],
                                                    rp_j[:])
                            nc.gpsimd.collective_compute(
                                "AllReduce", ALU.add, replica_groups=GROUPS,
                                ins=[rin.opt()], outs=[rout.opt()])
                            r_red = rp.tile([128, 8 * n], F32, tag="rred",
                                            name="rred")
                            nc.gpsimd.dma_start(r_red[:], rout[:])
                            rsrcs = [r_red[:, 8 * j:8 * j + 8]
                                     for j in range(n)]
                        rinv = rp.tile([128, 4 * n], F32, tag="rinv",
                                       name="rinv")
                        for j in range(n):
                            nc.vector.tensor_add(
                                rinv[:, 4 * j:4 * j + 4],
                                rsrcs[j][:, 0:4], rsrcs[j][:, 4:8])
                        nc.vector.reciprocal(rinv[:], rinv[:])
                        for j, (kc2, _, _) in enumerate(pend):
                            for s in range(NSUB):
                                g = kc2 * NSUB + s
                                nc.vector.tensor_scalar_mul(
                                    Vt[g][:], Vt[g][:],
                                    rinv[:, 4 * j + s:4 * j + s + 1])
                        # y += (V^T/r).T @ E: per-qt PSUM session over this
                        # group's chunks, then accumulate into y_sb.  The
                        # matmuls are queued and drained between future
                        # strips (see pending_y above).
                        def make_session(qt, group, gidx):
                            qsl = slice(qt * QT, (qt + 1) * QT)
                            state = {}
                            nmm = len(group) * NSUB
                            items = []

                            def mk_mm(mm, kc2, E2, s):
                                def emit():
                                    if mm == 0:
                                        state["yp"] = yps.tile(
                                            [128, QT], F32, tag="y",
                                            name="yps")
                                    nc.tensor.matmul(
                                        state["yp"][:],
                                        Vt[kc2 * NSUB + s][:],
                                        E2[s][:, qsl],
                                        start=(mm == 0),
                                        stop=(mm == nmm - 1))
                                return emit

                            mm = 0
                            for kc2, E2, _ in group:
                                for s in range(NSUB):
                                    items.append(mk_mm(mm, kc2, E2, s))
                                    mm += 1

                            def emit_evac():
                                if gidx == 0:
                                    nc.vector.tensor_copy(y_sb[qt][:],
                                                          state["yp"][:])
                                else:
                                    nc.vector.tensor_add(
                                        y_sb[qt][:],
                                        y_sb[qt][:].bitcast(F32),
                                        state["yp"][:])
                            items.append(emit_evac)
                            # mark only the final group's evacuations: the
                            # tail drain hangs each z projection off them
                            if gidx == 4:
                                items[-1].is_evac = qt
                            return items

                        for qt in range(4):
                            pending_y.extend(
                                make_session(qt, list(pend), group_idx))
                        group_idx += 1
                        pend = []
                    # drain remaining y work; as each qt's final
                    # accumulation lands, its output projection follows
                    # immediately so the tail pipeline stays full
                    with tc.tile_pool(name="zt", bufs=4) as ztp:
                        def emit_z(qt):
                            qsl = slice(qt * QT, (qt + 1) * QT)
                            for co in range(2):
                                ps = sps.tile([128, 1024], F32, tag="s",
                                              name="zps")
                                nc.tensor.matmul(
                                    ps[:, 0:QT],
                                    wzT_sb[:, co * 128:(co + 1) * 128],
                                    y_sb[qt][:], start=True, stop=True)
                                zt = ztp.tile([128, QT], F32, tag="zt",
                                              name="zt")
                                # z = (psum + bz) + xq
                                nc.vector.scalar_tensor_tensor(
                                    zt[:], ps[:, 0:QT], bz_sb[co][:],
                                    xq_sb[co][:, qsl].bitcast(F32),
                                    op0=ALU.add, op1=ALU.add)
                                nc.sync.dma_start(
                                    z_d[co * 128:(co + 1) * 128, qsl], zt[:])

                        while pending_y:
                            item = pending_y.pop(0)
                            item()
                            if getattr(item, "is_evac", None) is not None:
                                emit_z(item.is_evac)
                vtp_cm.__exit__(None, None, None)
                sps_cm.__exit__(None, None, None)

    nc.compile()
    return nc


def make_in_maps(inputs: dict) -> list:
    x = np.ascontiguousarray(np.asarray(inputs["x"], np.float32)
                             .reshape(B, C, N))
    aux = np.ascontiguousarray(np.asarray(inputs["aux"], np.float32)
                               .reshape(B, C, N))
    wqT = np.ascontiguousarray(np.asarray(inputs["wq_w"], np.float32).T)
    wkT = np.ascontiguousarray(np.asarray(inputs["wk_w"], np.float32).T)
    wvT = np.ascontiguousarray(np.asarray(inputs["wv_w"], np.float32).T)
    wzT = np.ascontiguousarray(np.asarray(inputs["wz_w"], np.float32).T)
    bq = np.asarray(inputs["wq_b"], np.float32).reshape(CH, 1)
    bk = np.asarray(inputs["wk_b"], np.float32).reshape(CH, 1)
    bv = np.asarray(inputs["wv_b"], np.float32).reshape(CH, 1)
    bz = np.asarray(inputs["wz_b"], np.float32).reshape(C, 1)
    ones = np.ones((1, CH), np.float32)
    in_maps = []
    for c in range(NCORES):
        b, h = c // 2, c % 2
        in_maps.append({
            "x": x[b],
            "xq": np.ascontiguousarray(x[b][:, h * NQ:(h + 1) * NQ]),
            "aux": aux[b],
            "wqT": wqT, "wkT": wkT, "wvT": wvT, "wzT": wzT,
            "bq": bq, "bk": bk, "bv": bv, "bz": bz, "ones": ones,
        })
    return in_maps


class Runner:
    """Compile once, then run the SPMD kernel any number of times.

    Mirrors bass2jax.run_bass_via_pjrt's multi-core branch but keeps the
    jitted executable so repeated calls don't re-trace/re-compile.
    """

    def __init__(self, f32r_attn: bool = F32R_ATTN,
                 f32r_proj: bool = F32R_PROJ, reps: int = 1, nc=None):
        import jax
        from jax.experimental.shard_map import shard_map
        from jax.sharding import Mesh, PartitionSpec

        self.nc = nc if nc is not None else build_program(
            f32r_attn, f32r_proj, reps=reps)
        bass2jax.install_neuronx_cc_hook()
        nc = self.nc
        assert nc.dbg_addr is None
        partition_name = (nc.partition_id_tensor.name
                          if nc.partition_id_tensor else None)

        in_names, out_names, out_avals, zero_outs = [], [], [], []
        for alloc in nc.m.functions[0].allocations:
            if not isinstance(alloc, mybir.MemoryLocationSet):
                continue
            name = alloc.memorylocations[0].name
            if alloc.kind == "ExternalInput":
                if name != partition_name:
                    in_names.append(name)
            elif alloc.kind == "ExternalOutput":
                out_names.append(name)
                shape = tuple(alloc.tensor_shape)
                dtype = mybir.dt.np(alloc.dtype)
                out_avals.append(jax.core.ShapedArray(shape, dtype))
                zero_outs.append(np.zeros(shape, dtype))
        self.in_names = list(in_names)
        self.out_names = out_names
        self.out_avals = out_avals
        n_params = len(in_names)
        n_outs = len(out_avals)
        all_names = in_names + out_names
        if partition_name is not None:
            all_names = all_names + [partition_name]

        def _body(*args):
            operands = list(args)
            if partition_name is not None:
                operands.append(bass2jax.partition_id_tensor())
            outs = bass2jax._bass_exec_p.bind(
                *operands,
                out_avals=tuple(out_avals),
                in_names=tuple(all_names),
                out_names=tuple(out_names),
                lowering_input_output_aliases=(),
                sim_require_finite=True,
                sim_require_nnan=True,
                nc=nc,
            )
            return tuple(outs)

        devices = jax.devices()[:NCORES]
        mesh = Mesh(np.asarray(devices), ("core",))
        from jax.sharding import NamedSharding
        self._sharding = NamedSharding(mesh, PartitionSpec("core"))
        in_specs = (PartitionSpec("core"),) * (n_params + n_outs)
        out_specs = (PartitionSpec("core"),) * n_outs
        self._sharded = jax.jit(
            shard_map(_body, mesh=mesh, in_specs=in_specs,
                      out_specs=out_specs, check_rep=False),
            donate_argnums=tuple(range(n_params, n_params + n_outs)),
            keep_unused=True,
        )
        self._zero_outs = zero_outs

    def device_inputs(self, in_maps):
        """Transfer the concatenated per-core inputs to the devices once."""
        import jax

        concat_in = [
            np.concatenate([np.asarray(in_maps[c][name])
                            for c in range(NCORES)], axis=0)
            for name in self.in_names
        ]
        return [jax.device_put(a, self._sharding) for a in concat_in]

    def run_device(self, dev_in):
        """Execute with device-resident inputs; returns device arrays."""
        concat_zeros = [
            np.zeros((NCORES * z.shape[0], *z.shape[1:]), z.dtype)
            for z in self._zero_outs
        ]
        return self._sharded(*dev_in, *concat_zeros)

    def run(self, in_maps):
        out_arrs = self.run_device(self.device_inputs(in_maps))
        return [
            {
                name: np.asarray(out_arrs[i]).reshape(
                    NCORES, *self.out_avals[i].shape)[c]
                for i, name in enumerate(self.out_names)
            }
            for c in range(NCORES)
        ]


_RUNNER = None


def get_runner() -> Runner:
    global _RUNNER
    if _RUNNER is None:
        _RUNNER = Runner()
    return _RUNNER


def assemble(results) -> np.ndarray:
    out = np.empty((B, C, N), np.float32)
    for c in range(NCORES):
        b, h = c // 2, c % 2
        out[b][:, h * NQ:(h + 1) * NQ] = results[c]["z"]
    return out.reshape(B, C, 64, 64)


def kernel(**inputs) -> np.ndarray:
    runner = get_runner()
    results = runner.run(make_in_maps(inputs))
    return assemble(results)

